# revision 26
# baseline (speedup 1.0000x reference)
"""FBANK kernel for Trainium2 (8 NeuronCores, pure data-parallel over batch).

Per core (8 batch rows): preemphasis folded into the DFT basis C (signal-level
equivalent since Hann w[0]=0), fp16 DFT-as-matmul with 4 K-chunks per frame
(128+128+128+16 samples) and a 78-col correction pass (split precision for fft
bins 1..32 + the 193..199 tail bins), squares on ACT into bf16, mel projection
as bf16 matmuls with bank-duplicated rows folding re^2+im^2, log+normalizer,
and a ragged masked-mean fixup via K=1/M=1 matmuls.
"""
import os
import numpy as np
import ml_dtypes

import concourse.bass as bass
import concourse.tile as tile
from concourse import mybir
from concourse.bass_utils import run_bass_kernel_spmd

BF16 = ml_dtypes.bfloat16
F16 = np.float16

SR, WIN, SHIFT, NMEL, PRE = 16000.0, 400, 160, 80, 0.97
EPS = float(np.finfo(np.float64).eps)
B, L = 64, 240000
F = 1 + (L - WIN) // SHIFT          # 1498
ROWS = 8                            # batch rows per core
NCORES = 8
G = 1504                            # padded frame-grid length
L_PAD = 240896                      # 160*1503 + 399 = 240879 max index
SPLIT = 32                          # fft bins 1..32 get split precision

TILES = [(0, 512), (512, 512), (1024, 474)]   # (f0, NF) per row

# k-chunks: (plane, g-offset, K). Plane row->sample maps (host):
#   A[v,g] = y[160g + v]            v in 0..127   (t = 0..127 @goff0, 160..287 @goff1)
#   C[v,g] = y[160g + 128+v] v<32;  y[160g + 256+v] v>=32   (t = 128..159, 288..383)
#   D[v,g] = y[160g + 384+v]        v in 0..15    (t = 384..399)
K_CHUNKS = [("A", 0, 128), ("A", 1, 128), ("C", 0, 128), ("D", 0, 16)]
# cp col layout (462): P1 [c1-32|s1-32|c33-96], P2 [c97-192|s33-64],
#                      P3 [s65-192], P4 [Cl c1-32|Cl s1-32|c193-199|s193-199]
# cb col layout (64):  [Ch c1-32|Ch s1-32]   (corrB: xl @ Ch, chunks A@0,A@1,C@0)
# x planes are DMA'd in per-f-tile column chunks so the first tile's matmuls
# start after ~1/3 of the plane transfer.
GCH = [(0, 515), (512, 515), (1024, 477)]     # (g0, width) per f-tile


def _build_consts():
    def hz2mel(f):
        return 1127.0 * np.log(1.0 + f / 700.0)
    mlow, mhigh = hz2mel(20.0), hz2mel(SR / 2.0)
    d = (mhigh - mlow) / (NMEL + 1)
    left = mlow + np.arange(NMEL) * d
    right = left + 2 * d
    fft_freqs = (SR / WIN) * np.arange(WIN // 2)
    mel = hz2mel(fft_freqs)[None, :]
    banks = np.maximum(0.0, np.minimum((mel - left[:, None]) / d,
                                       (right[:, None] - mel) / d))
    banks = np.concatenate([banks, np.zeros((NMEL, 1))], axis=1)  # (80, 201)
    bT = banks.T[1:200]                                           # (199, 80)

    w = 0.5 - 0.5 * np.cos(2 * np.pi * np.arange(WIN) / (WIN - 1))
    t_ = np.arange(WIN)
    k_ = np.arange(1, 200)
    ang = -2 * np.pi * np.outer(t_, k_) / WIN
    Cc = w[:, None] * np.cos(ang)          # (400, 199), preemph on signal
    Cs = w[:, None] * np.sin(ang)

    Ch_c = Cc.astype(F16)
    Ch_s = Cs.astype(F16)
    Cl_c = (Cc - Ch_c.astype(np.float64)).astype(F16)
    Cl_s = (Cs - Ch_s.astype(np.float64)).astype(F16)

    # full column blocks (400 rows), fp16
    cp_full = np.concatenate([
        Ch_c[:, 0:32], Ch_s[:, 0:32], Ch_c[:, 32:96],       # P1 (128)
        Ch_c[:, 96:192], Ch_s[:, 32:64],                    # P2 (128)
        Ch_s[:, 64:192],                                    # P3 (128)
        Cl_c[:, 0:32], Cl_s[:, 0:32],                       # P4 corr (64)
        Ch_c[:, 192:199], Ch_s[:, 192:199],                 # P4 M3 (14)
    ], axis=1).astype(F16)                                  # (400, 462)
    cb_full = np.concatenate([Ch_c[:, 0:32], Ch_s[:, 0:32]], axis=1)  # (400, 64)

    # k-chunk row maps
    tmaps = {
        0: np.arange(0, 128),
        1: np.arange(160, 288),
        2: np.concatenate([np.arange(128, 160), np.arange(288, 384)]),
        3: np.arange(384, 400),
    }
    cp = np.zeros((128, 4, 462), F16)
    cb = np.zeros((128, 3, 64), F16)
    for kc in range(4):
        t = tmaps[kc]
        cp[:len(t), kc] = cp_full[t]
        if kc < 3:
            cb[:len(t), kc] = cb_full[t]

    # mel bank tiles (bf16), rows follow sq partition layouts
    bd = np.zeros((128, 4, NMEL), BF16)
    bd[0:32, 0] = bT[0:32];  bd[32:64, 0] = bT[0:32];  bd[64:128, 0] = bT[32:96]
    bd[0:96, 1] = bT[96:192]; bd[96:128, 1] = bT[32:64]
    bd[:, 2] = bT[64:192]
    bd[0:7, 3] = bT[192:199]; bd[7:14, 3] = bT[192:199]
    return cp, cb, bd


_CP, _CB, _BD = _build_consts()
_NC = None


def _build_program():
    nc = bass.Bass("TRN2", target_bir_lowering=False, debug=False)
    dt = mybir.dt
    xa = nc.dram_tensor("xa", [ROWS, 128, G], dt.float16, kind="ExternalInput")
    xc = nc.dram_tensor("xc", [ROWS, 128, G], dt.float16, kind="ExternalInput")
    xd = nc.dram_tensor("xd", [ROWS, 16, G], dt.float16, kind="ExternalInput")
    la = nc.dram_tensor("la", [ROWS, 128, G], dt.float16, kind="ExternalInput")
    lc = nc.dram_tensor("lc", [ROWS, 128, G], dt.float16, kind="ExternalInput")
    cp = nc.dram_tensor("cp", [128, 4, 462], dt.float16, kind="ExternalInput")
    cb = nc.dram_tensor("cb", [128, 3, 64], dt.float16, kind="ExternalInput")
    bd = nc.dram_tensor("bd", [128, 4, NMEL], dt.bfloat16, kind="ExternalInput")
    nb = nc.dram_tensor("nb", [128, 4, NMEL], dt.float32, kind="ExternalInput")
    md = nc.dram_tensor("md", [128, ROWS], dt.float32, kind="ExternalInput")
    mb = nc.dram_tensor("mb", [1, ROWS * 128], dt.float32, kind="ExternalInput")
    # [row, frame%128, frame//128, mel]: one batched DMA per f-tile writes
    # [128, 4, 80]; host transposes back and drops frames 1498..1535
    out = nc.dram_tensor("out", [ROWS, 128, 12, NMEL], dt.float32,
                         kind="ExternalOutput")

    from contextlib import ExitStack
    with tile.TileContext(nc) as tc, ExitStack() as ctx:
        singles = ctx.enter_context(tc.tile_pool(name="singles", bufs=1))
        xpool = ctx.enter_context(tc.tile_pool(name="xpool", bufs=2))
        sqpool = ctx.enter_context(tc.tile_pool(name="sqpool", bufs=2))
        fpool = ctx.enter_context(tc.tile_pool(name="fpool", bufs=3))
        dftps = ctx.enter_context(tc.tile_pool(name="dftps", bufs=2, space="PSUM"))
        pcps = ctx.enter_context(tc.tile_pool(name="pcps", bufs=1, space="PSUM"))
        melps = ctx.enter_context(tc.tile_pool(name="melps", bufs=1, space="PSUM"))

        # consts on the ACT HWDGE queue so the SP queue starts on x planes
        # immediately; packed [128, n, cols] so each is one DMA
        cp0 = singles.tile([128, 462], dt.float16, tag="cp0")
        nc.scalar.dma_start(cp0[:], cp[:, 0, :])   # kc0 own tile: unblocks first LDW
        cpk = singles.tile([128, 3, 462], dt.float16, tag="cp")
        nc.scalar.dma_start(cpk[:], cp[:, 1:4, :])
        cbk = singles.tile([128, 3, 64], dt.float16, tag="cb")
        nc.scalar.dma_start(cbk[:], cb[:])
        bdk = singles.tile([128, 4, NMEL], dt.bfloat16, tag="bd")
        nc.scalar.dma_start(bdk[:], bd[:])
        cp_t = [cp0[:]] + [cpk[:, i, :] for i in range(3)]
        cb_t = [cbk[:, i, :] for i in range(3)]
        bd_t = [bdk[:, i, :] for i in range(4)]
        nb_t = singles.tile([128, 4, NMEL], dt.float32, tag="nb")
        nc.scalar.dma_start(nb_t[:], nb[:])
        md_t = singles.tile([128, ROWS], dt.float32, tag="md")
        nc.scalar.dma_start(md_t[:], md[:])
        mb_t = singles.tile([1, ROWS * 128], dt.float32, tag="mb")
        nc.scalar.dma_start(mb_t[:], mb[:])
        eps_t = singles.tile([128, 1], dt.float32, tag="eps")
        nc.vector.memset(eps_t[:], EPS)

        for r in range(ROWS):
            # row 0: per-f-tile chunked plane DMAs (fast start); later rows:
            # one DMA per plane (HWDGE queue costs ~625ns per DMA instruction)
            pchunks = []
            if r == 0:
                for ti, (g0, w) in enumerate(GCH):
                    xa_t = xpool.tile([128, w], dt.float16, tag=f"xa{ti}")
                    nc.sync.dma_start(out=xa_t[:], in_=xa[r, :, g0:g0 + w])
                    xc_t = xpool.tile([128, w], dt.float16, tag=f"xc{ti}")
                    nc.sync.dma_start(out=xc_t[:], in_=xc[r, :, g0:g0 + w])
                    xd_t = xpool.tile([16, w], dt.float16, tag=f"xd{ti}")
                    nc.sync.dma_start(out=xd_t[:], in_=xd[r, :, g0:g0 + w])
                    la_t = xpool.tile([128, w], dt.float16, tag=f"la{ti}")
                    nc.sync.dma_start(out=la_t[:], in_=la[r, :, g0:g0 + w])
                    lc_t = xpool.tile([128, w], dt.float16, tag=f"lc{ti}")
                    nc.sync.dma_start(out=lc_t[:], in_=lc[r, :, g0:g0 + w])
                    pchunks.append(({"A": xa_t, "C": xc_t, "D": xd_t},
                                    [(la_t, 0), (la_t, 1), (lc_t, 0)], 0))
            else:
                xa_t = xpool.tile([128, G], dt.float16, tag="xaf")
                nc.sync.dma_start(out=xa_t[:], in_=xa[r])
                xc_t = xpool.tile([128, G], dt.float16, tag="xcf")
                nc.sync.dma_start(out=xc_t[:], in_=xc[r])
                xd_t = xpool.tile([16, G], dt.float16, tag="xdf")
                nc.sync.dma_start(out=xd_t[:], in_=xd[r])
                la_t = xpool.tile([128, G], dt.float16, tag="laf")
                nc.sync.dma_start(out=la_t[:], in_=la[r])
                lc_t = xpool.tile([128, G], dt.float16, tag="lcf")
                nc.sync.dma_start(out=lc_t[:], in_=lc[r])

            row_tiles = TILES
            if r > 0:
                pchunks = [({"A": xa_t, "C": xc_t, "D": xd_t},
                            [(la_t, 0), (la_t, 1), (lc_t, 0)], f0)
                           for (f0, NF) in row_tiles]

            for ti, (f0, NF) in enumerate(row_tiles):
                planes, lplanes, gb = pchunks[ti]
                b1 = dftps.tile([128, 512], dt.float32, tag="b1")
                b2 = dftps.tile([128, 512], dt.float32, tag="b2")
                b3 = dftps.tile([128, 512], dt.float32, tag="b3")
                pc = pcps.tile([78, 512], dt.float32, tag="pc")

                # hi passes P1..P3 into b1..b3
                for mi, breg in enumerate((b1, b2, b3)):
                    lo = mi * 128
                    for kc, (pl, goff, K) in enumerate(K_CHUNKS):
                        nc.tensor.matmul(
                            breg[:, 0:NF],
                            cp_t[kc][0:K, lo:lo + 128],
                            planes[pl][0:K, gb + goff:gb + goff + NF],
                            start=(kc == 0), stop=(kc == 3))
                # P4 (corrA + M3) and corrB into pc; kc3 last closes the group
                for kc in (0, 1, 2):
                    pl, goff, K = K_CHUNKS[kc]
                    nc.tensor.matmul(
                        pc[0:78, 0:NF], cp_t[kc][0:K, 384:462],
                        planes[pl][0:K, gb + goff:gb + goff + NF],
                        start=(kc == 0), stop=False)
                for i, (lt, goff) in enumerate(lplanes):
                    nc.tensor.matmul(
                        pc[0:64, 0:NF], cb_t[i][0:128, 0:64],
                        lt[0:128, gb + goff:gb + goff + NF],
                        start=False, stop=False)
                pl, goff, K = K_CHUNKS[3]
                nc.tensor.matmul(
                    pc[0:78, 0:NF], cp_t[3][0:K, 384:462],
                    planes[pl][0:K, gb + goff:gb + goff + NF],
                    start=False, stop=True)

                # fold split-precision correction into b1 cols 0..63
                # (DVE reads at most one PSUM input -> stage via SBUF)
                cs = fpool.tile([64, 512], dt.float32, tag="cs")
                nc.vector.tensor_copy(cs[:, 0:NF], pc[0:64, 0:NF])
                nc.vector.tensor_add(b1[0:64, 0:NF], b1[0:64, 0:NF],
                                     cs[:, 0:NF])

                # squares (psum fp32 -> sbuf bf16)
                sq1 = sqpool.tile([128, 512], dt.bfloat16, tag="sq1")
                sq2 = sqpool.tile([128, 512], dt.bfloat16, tag="sq2")
                sq3 = sqpool.tile([128, 512], dt.bfloat16, tag="sq3")
                sqm = sqpool.tile([14, 512], dt.bfloat16, tag="sqm")
                SQ = mybir.ActivationFunctionType.Square
                nc.scalar.activation(sq1[:, 0:NF], b1[:, 0:NF], SQ)
                nc.scalar.activation(sq2[:, 0:NF], b2[:, 0:NF], SQ)
                nc.scalar.activation(sq3[:, 0:NF], b3[:, 0:NF], SQ)
                nc.scalar.activation(sqm[0:14, 0:NF], pc[64:78, 0:NF], SQ)

                # mel matmuls: out[frame, mel]; [128,6,80] psum tile also
                # hosts the fixup regions (j=4: fix, j=5 row 0: mean)
                mps = melps.tile([128, 6, NMEL], dt.float32, tag="mps")
                nsub = (NF + 127) // 128
                for j in range(nsub):
                    nj = min(128, NF - j * 128)
                    fr = slice(j * 128, j * 128 + nj)
                    chunks = [(sq1, 128, bd_t[0]), (sq2, 128, bd_t[1]),
                              (sq3, 128, bd_t[2]), (sqm, 14, bd_t[3])]
                    for ci, (sqt, K, bdt) in enumerate(chunks):
                        nc.tensor.matmul(mps[0:nj, j, :], sqt[0:K, fr],
                                         bdt[0:K, :],
                                         start=(ci == 0), stop=(ci == 3))

                # log(mel + EPS) ~= log(max(mel, EPS)): EPS shifts mel by
                # <1e-8 relative at the observed magnitudes
                out_t = fpool.tile([128, 4, NMEL], dt.float32, tag="out")
                nfull, rem = NF // 128, NF % 128
                views = []
                if nfull:
                    views.append((slice(0, 128), slice(0, nfull)))
                if rem:
                    views.append((slice(0, rem), slice(nfull, nfull + 1)))
                for pv, jv in views:
                    nc.scalar.activation(out_t[pv, jv, :], mps[pv, jv, :],
                                         mybir.ActivationFunctionType.Ln,
                                         bias=eps_t[pv])
                    nc.vector.tensor_mul(out_t[pv, jv, :], out_t[pv, jv, :],
                                         nb_t[pv, jv, :])

                # ragged masked-mean fixup (masked frames all lie in f < 128)
                if f0 == 0:
                    nc.tensor.matmul(mps[0:1, 5, :], md_t[:, r:r + 1],
                                     out_t[:, 0, :], start=True, stop=True)
                    mean_sb = fpool.tile([1, NMEL], dt.float32, tag="mean_sb")
                    nc.vector.tensor_copy(mean_sb[:], mps[0:1, 5, :])
                    nc.tensor.matmul(mps[:, 4, :],
                                     mb_t[0:1, r * 128:(r + 1) * 128],
                                     mean_sb[:], start=True, stop=True)
                    nc.vector.tensor_sub(out_t[:, 0, :], out_t[:, 0, :],
                                         mps[:, 4, :])

                # store: one batched DMA per tile on the ACT HWDGE queue
                j0 = f0 // 128
                nc.scalar.dma_start(out[r, :, j0:j0 + nsub, :],
                                    out_t[:, 0:nsub, :])
    import bass_rust
    bass_rust.generate_event_semaphores(nc)   # split multi-waits for walrus codegen
    return nc


def _plane(src, off, n):
    s = src.strides
    v = np.lib.stride_tricks.as_strided(
        src[:, off:], shape=(B, n, G), strides=(s[0], s[1], 160 * s[1]))
    return v


def _host_prep(x, T, normalizer):
    xf = np.asarray(x, np.float32)
    # signal-level preemphasis (valid: Hann w[0] = 0 kills the frame-edge term)
    y = np.empty((B, L_PAD), np.float32)
    y[:, 0] = xf[:, 0] * (1.0 - PRE)
    y[:, 1:L] = xf[:, 1:] - PRE * xf[:, :-1]
    y[:, L:] = 0.0
    yh = y.astype(F16)
    yl = (y - yh.astype(np.float32)).astype(F16)

    xa = np.ascontiguousarray(_plane(yh, 0, 128))
    xcp = np.empty((B, 128, G), F16)
    xcp[:, 0:32] = _plane(yh, 128, 32)
    xcp[:, 32:128] = _plane(yh, 288, 96)
    xd = np.ascontiguousarray(_plane(yh, 384, 16))
    la = np.ascontiguousarray(_plane(yl, 0, 128))
    lcp = np.empty((B, 128, G), F16)
    lcp[:, 0:32] = _plane(yl, 128, 32)
    lcp[:, 32:128] = _plane(yl, 288, 96)

    T = np.asarray(T, np.int32)
    ds = T.max().astype(np.float32) / np.float32(NMEL)
    T_ = (T.astype(np.float32) / ds).astype(np.int32)
    cnt = np.maximum(T_, 1).astype(np.float32)
    f = np.arange(128)[None, :]
    maskbit = (f < T_[:, None]).astype(np.float32)          # (64, 128)
    maskdiv = maskbit / cnt[:, None]

    nrm = np.asarray(normalizer, np.float32)
    nb = np.broadcast_to(nrm[None, None, :], (128, 4, NMEL)).copy()
    return xa, xcp, xd, la, lcp, maskdiv, maskbit, nb


def _bass_kernel(x, T, normalizer):
    global _NC
    if _NC is None:
        _NC = _build_program()
    xa, xcp, xd, la, lcp, maskdiv, maskbit, nb = _host_prep(x, T, normalizer)
    in_maps = []
    for c in range(NCORES):
        r0 = c * ROWS
        in_maps.append({
            "xa": xa[r0:r0 + ROWS], "xc": xcp[r0:r0 + ROWS],
            "xd": xd[r0:r0 + ROWS], "la": la[r0:r0 + ROWS],
            "lc": lcp[r0:r0 + ROWS],
            "cp": _CP, "cb": _CB, "bd": _BD, "nb": nb,
            "md": np.ascontiguousarray(maskdiv[r0:r0 + ROWS].T),
            "mb": maskbit[r0:r0 + ROWS].reshape(1, -1),
        })
    trace = bool(int(os.environ.get("KERNEL_TRACE", "0")))
    res = run_bass_kernel_spmd(_NC, in_maps, list(range(NCORES)), trace=trace)
    if res.exec_time_ns is not None:
        print(f"HW exec time: {res.exec_time_ns} ns")
    full = np.concatenate([res.results[c]["out"] for c in range(NCORES)], axis=0)
    # [B, 128, 12, 80] -> [B, 1536, 80] -> drop padded frames
    return np.ascontiguousarray(
        full.transpose(0, 2, 1, 3).reshape(B, 1536, NMEL)[:, :F, :])


def kernel(x, T, normalizer):
    return _bass_kernel(x, T, normalizer)


# revision 27
# speedup vs baseline: 1.0044x; 1.0044x over previous
"""FBANK kernel for Trainium2 (8 NeuronCores, pure data-parallel over batch).

Per core (8 batch rows): preemphasis folded into the DFT basis C (signal-level
equivalent since Hann w[0]=0), fp16 DFT-as-matmul with 4 K-chunks per frame
(128+128+128+16 samples) and a 78-col correction pass (split precision for fft
bins 1..32 + the 193..199 tail bins), squares on ACT into bf16, mel projection
as bf16 matmuls with bank-duplicated rows folding re^2+im^2, log+normalizer,
and a ragged masked-mean fixup via K=1/M=1 matmuls.
"""
import os
import numpy as np
import ml_dtypes

import concourse.bass as bass
import concourse.tile as tile
from concourse import mybir
from concourse.bass_utils import run_bass_kernel_spmd

BF16 = ml_dtypes.bfloat16
F16 = np.float16

SR, WIN, SHIFT, NMEL, PRE = 16000.0, 400, 160, 80, 0.97
EPS = float(np.finfo(np.float64).eps)
B, L = 64, 240000
F = 1 + (L - WIN) // SHIFT          # 1498
ROWS = 8                            # batch rows per core
NCORES = 8
G = 1504                            # padded frame-grid length
L_PAD = 240896                      # 160*1503 + 399 = 240879 max index
SPLIT = 32                          # fft bins 1..32 get split precision

TILES = [(0, 512), (512, 512), (1024, 474)]   # (f0, NF) per row

# k-chunks: (plane, g-offset, K). Plane row->sample maps (host):
#   A[v,g] = y[160g + v]            v in 0..127   (t = 0..127 @goff0, 160..287 @goff1)
#   C[v,g] = y[160g + 128+v] v<32;  y[160g + 256+v] v>=32   (t = 128..159, 288..383)
#   D[v,g] = y[160g + 384+v]        v in 0..15    (t = 384..399)
K_CHUNKS = [("A", 0, 128), ("A", 1, 128), ("C", 0, 128), ("D", 0, 16)]
# cp col layout (462): P1 [c1-32|s1-32|c33-96], P2 [c97-192|s33-64],
#                      P3 [s65-192], P4 [Cl c1-32|Cl s1-32|c193-199|s193-199]
# cb col layout (64):  [Ch c1-32|Ch s1-32]   (corrB: xl @ Ch, chunks A@0,A@1,C@0)
# x planes are DMA'd in per-f-tile column chunks so the first tile's matmuls
# start after ~1/3 of the plane transfer.
GCH = [(0, 515), (512, 515), (1024, 477)]     # (g0, width) per f-tile


def _build_consts():
    def hz2mel(f):
        return 1127.0 * np.log(1.0 + f / 700.0)
    mlow, mhigh = hz2mel(20.0), hz2mel(SR / 2.0)
    d = (mhigh - mlow) / (NMEL + 1)
    left = mlow + np.arange(NMEL) * d
    right = left + 2 * d
    fft_freqs = (SR / WIN) * np.arange(WIN // 2)
    mel = hz2mel(fft_freqs)[None, :]
    banks = np.maximum(0.0, np.minimum((mel - left[:, None]) / d,
                                       (right[:, None] - mel) / d))
    banks = np.concatenate([banks, np.zeros((NMEL, 1))], axis=1)  # (80, 201)
    bT = banks.T[1:200]                                           # (199, 80)

    w = 0.5 - 0.5 * np.cos(2 * np.pi * np.arange(WIN) / (WIN - 1))
    t_ = np.arange(WIN)
    k_ = np.arange(1, 200)
    ang = -2 * np.pi * np.outer(t_, k_) / WIN
    Cc = w[:, None] * np.cos(ang)          # (400, 199), preemph on signal
    Cs = w[:, None] * np.sin(ang)

    Ch_c = Cc.astype(F16)
    Ch_s = Cs.astype(F16)
    Cl_c = (Cc - Ch_c.astype(np.float64)).astype(F16)
    Cl_s = (Cs - Ch_s.astype(np.float64)).astype(F16)

    # full column blocks (400 rows), fp16
    cp_full = np.concatenate([
        Ch_c[:, 0:32], Ch_s[:, 0:32], Ch_c[:, 32:96],       # P1 (128)
        Ch_c[:, 96:192], Ch_s[:, 32:64],                    # P2 (128)
        Ch_s[:, 64:192],                                    # P3 (128)
        Cl_c[:, 0:32], Cl_s[:, 0:32],                       # P4 corr (64)
        Ch_c[:, 192:199], Ch_s[:, 192:199],                 # P4 M3 (14)
    ], axis=1).astype(F16)                                  # (400, 462)
    cb_full = np.concatenate([Ch_c[:, 0:32], Ch_s[:, 0:32]], axis=1)  # (400, 64)

    # k-chunk row maps
    tmaps = {
        0: np.arange(0, 128),
        1: np.arange(160, 288),
        2: np.concatenate([np.arange(128, 160), np.arange(288, 384)]),
        3: np.arange(384, 400),
    }
    cp = np.zeros((128, 4, 462), F16)
    cb = np.zeros((128, 3, 64), F16)
    for kc in range(4):
        t = tmaps[kc]
        cp[:len(t), kc] = cp_full[t]
        if kc < 3:
            cb[:len(t), kc] = cb_full[t]

    # mel bank tiles (bf16), rows follow sq partition layouts
    bd = np.zeros((128, 4, NMEL), BF16)
    bd[0:32, 0] = bT[0:32];  bd[32:64, 0] = bT[0:32];  bd[64:128, 0] = bT[32:96]
    bd[0:96, 1] = bT[96:192]; bd[96:128, 1] = bT[32:64]
    bd[:, 2] = bT[64:192]
    bd[0:7, 3] = bT[192:199]; bd[7:14, 3] = bT[192:199]
    return cp, cb, bd


_CP, _CB, _BD = _build_consts()
_NC = None


def _build_program():
    nc = bass.Bass("TRN2", target_bir_lowering=False, debug=False)
    dt = mybir.dt
    xa = nc.dram_tensor("xa", [ROWS, 128, G], dt.float16, kind="ExternalInput")
    xc = nc.dram_tensor("xc", [ROWS, 128, G], dt.float16, kind="ExternalInput")
    xd = nc.dram_tensor("xd", [ROWS, 16, G], dt.float16, kind="ExternalInput")
    la = nc.dram_tensor("la", [ROWS, 128, G], dt.float16, kind="ExternalInput")
    lc = nc.dram_tensor("lc", [ROWS, 128, G], dt.float16, kind="ExternalInput")
    cp = nc.dram_tensor("cp", [128, 4, 462], dt.float16, kind="ExternalInput")
    cb = nc.dram_tensor("cb", [128, 3, 64], dt.float16, kind="ExternalInput")
    bd = nc.dram_tensor("bd", [128, 4, NMEL], dt.bfloat16, kind="ExternalInput")
    nb = nc.dram_tensor("nb", [128, 4, NMEL], dt.float32, kind="ExternalInput")
    md = nc.dram_tensor("md", [128, ROWS], dt.float32, kind="ExternalInput")
    mb = nc.dram_tensor("mb", [1, ROWS * 128], dt.float32, kind="ExternalInput")
    # [row, frame%128, frame//128, mel]: one batched DMA per f-tile writes
    # [128, 4, 80]; host transposes back and drops frames 1498..1535
    out = nc.dram_tensor("out", [ROWS, 128, 12, NMEL], dt.float32,
                         kind="ExternalOutput")

    from contextlib import ExitStack
    with tile.TileContext(nc) as tc, ExitStack() as ctx:
        singles = ctx.enter_context(tc.tile_pool(name="singles", bufs=1))
        xpool = ctx.enter_context(tc.tile_pool(name="xpool", bufs=3))
        sqpool = ctx.enter_context(tc.tile_pool(name="sqpool", bufs=3))
        fpool = ctx.enter_context(tc.tile_pool(name="fpool", bufs=4))
        dftps = ctx.enter_context(tc.tile_pool(name="dftps", bufs=2, space="PSUM"))
        pcps = ctx.enter_context(tc.tile_pool(name="pcps", bufs=1, space="PSUM"))
        melps = ctx.enter_context(tc.tile_pool(name="melps", bufs=1, space="PSUM"))

        # consts on the ACT HWDGE queue so the SP queue starts on x planes
        # immediately; packed [128, n, cols] so each is one DMA
        cp0 = singles.tile([128, 462], dt.float16, tag="cp0")
        nc.scalar.dma_start(cp0[:], cp[:, 0, :])   # kc0 own tile: unblocks first LDW
        cpk = singles.tile([128, 3, 462], dt.float16, tag="cp")
        nc.scalar.dma_start(cpk[:], cp[:, 1:4, :])
        cbk = singles.tile([128, 3, 64], dt.float16, tag="cb")
        nc.scalar.dma_start(cbk[:], cb[:])
        bdk = singles.tile([128, 4, NMEL], dt.bfloat16, tag="bd")
        nc.scalar.dma_start(bdk[:], bd[:])
        cp_t = [cp0[:]] + [cpk[:, i, :] for i in range(3)]
        cb_t = [cbk[:, i, :] for i in range(3)]
        bd_t = [bdk[:, i, :] for i in range(4)]
        nb_t = singles.tile([128, 4, NMEL], dt.float32, tag="nb")
        nc.scalar.dma_start(nb_t[:], nb[:])
        md_t = singles.tile([128, ROWS], dt.float32, tag="md")
        nc.scalar.dma_start(md_t[:], md[:])
        mb_t = singles.tile([1, ROWS * 128], dt.float32, tag="mb")
        nc.scalar.dma_start(mb_t[:], mb[:])
        eps_t = singles.tile([128, 1], dt.float32, tag="eps")
        nc.vector.memset(eps_t[:], EPS)

        for r in range(ROWS):
            # row 0: per-f-tile chunked plane DMAs (fast start); later rows:
            # one DMA per plane (HWDGE queue costs ~625ns per DMA instruction)
            pchunks = []
            if r == 0:
                for ti, (g0, w) in enumerate(GCH):
                    xa_t = xpool.tile([128, w], dt.float16, tag=f"xa{ti}")
                    nc.sync.dma_start(out=xa_t[:], in_=xa[r, :, g0:g0 + w])
                    xc_t = xpool.tile([128, w], dt.float16, tag=f"xc{ti}")
                    nc.sync.dma_start(out=xc_t[:], in_=xc[r, :, g0:g0 + w])
                    xd_t = xpool.tile([16, w], dt.float16, tag=f"xd{ti}")
                    nc.sync.dma_start(out=xd_t[:], in_=xd[r, :, g0:g0 + w])
                    la_t = xpool.tile([128, w], dt.float16, tag=f"la{ti}")
                    nc.sync.dma_start(out=la_t[:], in_=la[r, :, g0:g0 + w])
                    lc_t = xpool.tile([128, w], dt.float16, tag=f"lc{ti}")
                    nc.sync.dma_start(out=lc_t[:], in_=lc[r, :, g0:g0 + w])
                    pchunks.append(({"A": xa_t, "C": xc_t, "D": xd_t},
                                    [(la_t, 0), (la_t, 1), (lc_t, 0)], 0))
            else:
                xa_t = xpool.tile([128, G], dt.float16, tag="xaf")
                nc.sync.dma_start(out=xa_t[:], in_=xa[r])
                xc_t = xpool.tile([128, G], dt.float16, tag="xcf")
                nc.sync.dma_start(out=xc_t[:], in_=xc[r])
                xd_t = xpool.tile([16, G], dt.float16, tag="xdf")
                nc.sync.dma_start(out=xd_t[:], in_=xd[r])
                la_t = xpool.tile([128, G], dt.float16, tag="laf")
                nc.sync.dma_start(out=la_t[:], in_=la[r])
                lc_t = xpool.tile([128, G], dt.float16, tag="lcf")
                nc.sync.dma_start(out=lc_t[:], in_=lc[r])

            row_tiles = TILES
            if r > 0:
                pchunks = [({"A": xa_t, "C": xc_t, "D": xd_t},
                            [(la_t, 0), (la_t, 1), (lc_t, 0)], f0)
                           for (f0, NF) in row_tiles]

            for ti, (f0, NF) in enumerate(row_tiles):
                planes, lplanes, gb = pchunks[ti]
                b1 = dftps.tile([128, 512], dt.float32, tag="b1")
                b2 = dftps.tile([128, 512], dt.float32, tag="b2")
                b3 = dftps.tile([128, 512], dt.float32, tag="b3")
                pc = pcps.tile([78, 512], dt.float32, tag="pc")

                # hi passes P1..P3 into b1..b3
                for mi, breg in enumerate((b1, b2, b3)):
                    lo = mi * 128
                    for kc, (pl, goff, K) in enumerate(K_CHUNKS):
                        nc.tensor.matmul(
                            breg[:, 0:NF],
                            cp_t[kc][0:K, lo:lo + 128],
                            planes[pl][0:K, gb + goff:gb + goff + NF],
                            start=(kc == 0), stop=(kc == 3))
                # P4 (corrA + M3) and corrB into pc; kc3 last closes the group
                for kc in (0, 1, 2):
                    pl, goff, K = K_CHUNKS[kc]
                    nc.tensor.matmul(
                        pc[0:78, 0:NF], cp_t[kc][0:K, 384:462],
                        planes[pl][0:K, gb + goff:gb + goff + NF],
                        start=(kc == 0), stop=False)
                for i, (lt, goff) in enumerate(lplanes):
                    nc.tensor.matmul(
                        pc[0:64, 0:NF], cb_t[i][0:128, 0:64],
                        lt[0:128, gb + goff:gb + goff + NF],
                        start=False, stop=False)
                pl, goff, K = K_CHUNKS[3]
                nc.tensor.matmul(
                    pc[0:78, 0:NF], cp_t[3][0:K, 384:462],
                    planes[pl][0:K, gb + goff:gb + goff + NF],
                    start=False, stop=True)

                # fold split-precision correction into b1 cols 0..63
                # (DVE reads at most one PSUM input -> stage via SBUF)
                cs = fpool.tile([64, 512], dt.float32, tag="cs")
                nc.vector.tensor_copy(cs[:, 0:NF], pc[0:64, 0:NF])
                nc.vector.tensor_add(b1[0:64, 0:NF], b1[0:64, 0:NF],
                                     cs[:, 0:NF])

                # squares (psum fp32 -> sbuf bf16)
                sq1 = sqpool.tile([128, 512], dt.bfloat16, tag="sq1")
                sq2 = sqpool.tile([128, 512], dt.bfloat16, tag="sq2")
                sq3 = sqpool.tile([128, 512], dt.bfloat16, tag="sq3")
                sqm = sqpool.tile([14, 512], dt.bfloat16, tag="sqm")
                SQ = mybir.ActivationFunctionType.Square
                nc.scalar.activation(sq1[:, 0:NF], b1[:, 0:NF], SQ)
                nc.scalar.activation(sq2[:, 0:NF], b2[:, 0:NF], SQ)
                nc.scalar.activation(sq3[:, 0:NF], b3[:, 0:NF], SQ)
                nc.scalar.activation(sqm[0:14, 0:NF], pc[64:78, 0:NF], SQ)

                # mel matmuls: out[frame, mel]; [128,6,80] psum tile also
                # hosts the fixup regions (j=4: fix, j=5 row 0: mean)
                mps = melps.tile([128, 6, NMEL], dt.float32, tag="mps")
                nsub = (NF + 127) // 128
                for j in range(nsub):
                    nj = min(128, NF - j * 128)
                    fr = slice(j * 128, j * 128 + nj)
                    chunks = [(sq1, 128, bd_t[0]), (sq2, 128, bd_t[1]),
                              (sq3, 128, bd_t[2]), (sqm, 14, bd_t[3])]
                    for ci, (sqt, K, bdt) in enumerate(chunks):
                        nc.tensor.matmul(mps[0:nj, j, :], sqt[0:K, fr],
                                         bdt[0:K, :],
                                         start=(ci == 0), stop=(ci == 3))

                # log(mel + EPS) ~= log(max(mel, EPS)): EPS shifts mel by
                # <1e-8 relative at the observed magnitudes
                out_t = fpool.tile([128, 4, NMEL], dt.float32, tag="out")
                nfull, rem = NF // 128, NF % 128
                views = []
                if nfull:
                    views.append((slice(0, 128), slice(0, nfull)))
                if rem:
                    views.append((slice(0, rem), slice(nfull, nfull + 1)))
                for pv, jv in views:
                    nc.scalar.activation(out_t[pv, jv, :], mps[pv, jv, :],
                                         mybir.ActivationFunctionType.Ln,
                                         bias=eps_t[pv])
                    nc.vector.tensor_mul(out_t[pv, jv, :], out_t[pv, jv, :],
                                         nb_t[pv, jv, :])

                # ragged masked-mean fixup (masked frames all lie in f < 128)
                if f0 == 0:
                    nc.tensor.matmul(mps[0:1, 5, :], md_t[:, r:r + 1],
                                     out_t[:, 0, :], start=True, stop=True)
                    mean_sb = fpool.tile([1, NMEL], dt.float32, tag="mean_sb")
                    nc.vector.tensor_copy(mean_sb[:], mps[0:1, 5, :])
                    nc.tensor.matmul(mps[:, 4, :],
                                     mb_t[0:1, r * 128:(r + 1) * 128],
                                     mean_sb[:], start=True, stop=True)
                    nc.vector.tensor_sub(out_t[:, 0, :], out_t[:, 0, :],
                                         mps[:, 4, :])

                # store: one batched DMA per tile on the ACT HWDGE queue
                j0 = f0 // 128
                nc.scalar.dma_start(out[r, :, j0:j0 + nsub, :],
                                    out_t[:, 0:nsub, :])
    import bass_rust
    bass_rust.generate_event_semaphores(nc)   # split multi-waits for walrus codegen
    return nc


def _plane(src, off, n):
    s = src.strides
    v = np.lib.stride_tricks.as_strided(
        src[:, off:], shape=(B, n, G), strides=(s[0], s[1], 160 * s[1]))
    return v


def _host_prep(x, T, normalizer):
    xf = np.asarray(x, np.float32)
    # signal-level preemphasis (valid: Hann w[0] = 0 kills the frame-edge term)
    y = np.empty((B, L_PAD), np.float32)
    y[:, 0] = xf[:, 0] * (1.0 - PRE)
    y[:, 1:L] = xf[:, 1:] - PRE * xf[:, :-1]
    y[:, L:] = 0.0
    yh = y.astype(F16)
    yl = (y - yh.astype(np.float32)).astype(F16)

    xa = np.ascontiguousarray(_plane(yh, 0, 128))
    xcp = np.empty((B, 128, G), F16)
    xcp[:, 0:32] = _plane(yh, 128, 32)
    xcp[:, 32:128] = _plane(yh, 288, 96)
    xd = np.ascontiguousarray(_plane(yh, 384, 16))
    la = np.ascontiguousarray(_plane(yl, 0, 128))
    lcp = np.empty((B, 128, G), F16)
    lcp[:, 0:32] = _plane(yl, 128, 32)
    lcp[:, 32:128] = _plane(yl, 288, 96)

    T = np.asarray(T, np.int32)
    ds = T.max().astype(np.float32) / np.float32(NMEL)
    T_ = (T.astype(np.float32) / ds).astype(np.int32)
    cnt = np.maximum(T_, 1).astype(np.float32)
    f = np.arange(128)[None, :]
    maskbit = (f < T_[:, None]).astype(np.float32)          # (64, 128)
    maskdiv = maskbit / cnt[:, None]

    nrm = np.asarray(normalizer, np.float32)
    nb = np.broadcast_to(nrm[None, None, :], (128, 4, NMEL)).copy()
    return xa, xcp, xd, la, lcp, maskdiv, maskbit, nb


def _bass_kernel(x, T, normalizer):
    global _NC
    if _NC is None:
        _NC = _build_program()
    xa, xcp, xd, la, lcp, maskdiv, maskbit, nb = _host_prep(x, T, normalizer)
    in_maps = []
    for c in range(NCORES):
        r0 = c * ROWS
        in_maps.append({
            "xa": xa[r0:r0 + ROWS], "xc": xcp[r0:r0 + ROWS],
            "xd": xd[r0:r0 + ROWS], "la": la[r0:r0 + ROWS],
            "lc": lcp[r0:r0 + ROWS],
            "cp": _CP, "cb": _CB, "bd": _BD, "nb": nb,
            "md": np.ascontiguousarray(maskdiv[r0:r0 + ROWS].T),
            "mb": maskbit[r0:r0 + ROWS].reshape(1, -1),
        })
    trace = bool(int(os.environ.get("KERNEL_TRACE", "0")))
    res = run_bass_kernel_spmd(_NC, in_maps, list(range(NCORES)), trace=trace)
    if res.exec_time_ns is not None:
        print(f"HW exec time: {res.exec_time_ns} ns")
    full = np.concatenate([res.results[c]["out"] for c in range(NCORES)], axis=0)
    # [B, 128, 12, 80] -> [B, 1536, 80] -> drop padded frames
    return np.ascontiguousarray(
        full.transpose(0, 2, 1, 3).reshape(B, 1536, NMEL)[:, :F, :])


def kernel(x, T, normalizer):
    return _bass_kernel(x, T, normalizer)
